# revision 8
# baseline (speedup 1.0000x reference)
import os
import sys

import numpy as np

if "/opt/trn_rl_repo" not in sys.path:
    sys.path.insert(0, "/opt/trn_rl_repo")

N = 2048          # atoms
NCORES = 8
NL = N // NCORES  # 256 rows of plm per core
C = 16            # c_atom_pair
P = 128           # partitions
C_IN, C_ATOM = 390, 128

_CACHE = {}


def _build_program():
    import concourse.bacc as bacc
    import concourse.bass as bass
    import concourse.mybir as mybir
    import concourse.tile as tile

    dt = mybir.dt.float32
    Alu = mybir.AluOpType
    Act = mybir.ActivationFunctionType

    nc = bacc.Bacc("TRN2", target_bir_lowering=False, debug=False,
                   num_devices=NCORES)

    # ---- DRAM I/O (per-core shapes; values differ per core) ----
    posm_b = nc.dram_tensor("posm_b", [3, P, N], dt, kind="ExternalInput")
    uid_b = nc.dram_tensor("uid_b", [P, N], dt, kind="ExternalInput")
    posl = nc.dram_tensor("posl", [NL, 3], dt, kind="ExternalInput")
    uidl = nc.dram_tensor("uidl", [NL, 1], dt, kind="ExternalInput")
    awt = nc.dram_tensor("awt", [C, N], dt, kind="ExternalInput")
    negaT = nc.dram_tensor("negaT", [C, NL], dt, kind="ExternalInput")
    dpat = nc.dram_tensor("dpat", [P, P * C], dt, kind="ExternalInput")
    epat = nc.dram_tensor("epat", [C, 512], dt, kind="ExternalInput")
    ones1 = nc.dram_tensor("ones1", [1, P], dt, kind="ExternalInput")
    eye = nc.dram_tensor("eye", [P, P], dt, kind="ExternalInput")
    featsT = nc.dram_tensor("featsT", [4, P, NL], dt, kind="ExternalInput")
    wfeats = nc.dram_tensor("wfeats", [4, P, C_ATOM], dt, kind="ExternalInput")

    plm_o = nc.dram_tensor("plm_o", [NL, N, C], dt, kind="ExternalOutput")
    cl_o = nc.dram_tensor("cl_o", [NL, C_ATOM], dt, kind="ExternalOutput")

    def ap3(base, dims):
        # rebuild an AP with explicit free-dim [step,count] list
        return bass.AP(base.tensor, base.offset, [list(base.ap[0])] + dims)

    with tile.TileContext(nc) as tc:
        with (
            tc.tile_pool(name="const", bufs=1) as cp,
            tc.tile_pool(name="lbp", bufs=2) as lp,
            tc.tile_pool(name="plane", bufs=1) as pp,
            tc.tile_pool(name="outp", bufs=3) as op_,
            tc.tile_pool(name="psum", bufs=2, space="PSUM") as ps,
        ):
            # ---- load constants ----
            t_posm = []
            for k in range(3):
                t = cp.tile([P, N], dt, tag=f"t_posm{k}")
                nc.sync.dma_start(t[:], posm_b[k, :, :])
                t_posm.append(t)
            t_uidb = cp.tile([P, N], dt, tag="t_uidb")
            nc.sync.dma_start(t_uidb[:], uid_b[:])
            t_negaT = cp.tile([C, NL], dt, tag="t_negaT")
            nc.sync.dma_start(t_negaT[:], negaT[:])
            t_D = cp.tile([P, P * C], dt, tag="t_D")
            nc.sync.dma_start(t_D[:], dpat[:])
            t_E = cp.tile([C, 512], dt, tag="t_E")
            nc.sync.dma_start(t_E[:], epat[:])
            t_ones = cp.tile([1, P], dt, tag="t_ones")
            nc.sync.dma_start(t_ones[:], ones1[:])
            t_eye = cp.tile([P, P], dt, tag="t_eye")
            nc.sync.dma_start(t_eye[:], eye[:])
            t_fT, t_wf = [], []
            for k in range(4):
                tf = cp.tile([P, NL], dt, tag=f"t_fT{k}")
                nc.sync.dma_start(tf[:], featsT[k, :, :])
                t_fT.append(tf)
                tw = cp.tile([P, C_ATOM], dt, tag=f"t_wf{k}")
                nc.sync.dma_start(tw[:], wfeats[k, :, :])
                t_wf.append(tw)

            # ---- cl = feats @ W_feats (tiny) ----
            for ab in range(NL // P):
                ps_cl = ps.tile([P, C_ATOM], dt, tag="pmain")
                for k in range(4):
                    nc.tensor.matmul(
                        ps_cl[:],
                        t_fT[k][:, ab * P:(ab + 1) * P],
                        t_wf[k][:],
                        start=(k == 0),
                        stop=(k == 3),
                    )
                sb_cl = op_.tile([P, C_ATOM], dt, tag="sb_cl")
                nc.scalar.copy(sb_cl[:], ps_cl[:])
                nc.sync.dma_start(cl_o[ab * P:(ab + 1) * P, :], sb_cl[:])

            # ---- plm ----
            for lb in range(NL // P):
                # per-partition scalars for this l-block
                t_posl = lp.tile([P, 3], dt, tag="t_posl")
                nc.sync.dma_start(t_posl[:], posl[lb * P:(lb + 1) * P, :])
                t_uidl = lp.tile([P, 1], dt, tag="t_uidl")
                nc.sync.dma_start(t_uidl[:], uidl[lb * P:(lb + 1) * P, :])

                # [l, m] planes: d2 -> d -> inv_d ; u ; w
                t_d = pp.tile([P, N], dt, tag="t_d")
                t_s = pp.tile([P, N], dt, tag="t_s")
                t_acc = pp.tile([P, N], dt, tag="t_acc")
                for k in range(3):
                    nc.vector.tensor_scalar_sub(
                        t_d[:], t_posm[k][:], t_posl[:, k:k + 1]
                    )
                    if k == 0:
                        nc.gpsimd.tensor_mul(t_acc[:], t_d[:], t_d[:])
                    else:
                        nc.gpsimd.tensor_mul(t_s[:], t_d[:], t_d[:])
                        nc.gpsimd.tensor_add(t_acc[:], t_acc[:], t_s[:])
                # d = exp(0.5*ln(d2)) ; inv_d = exp(-ln(1+d))  (diagonal: exact 1.0)
                nc.scalar.activation(t_s[:], t_acc[:], Act.Ln)
                nc.scalar.activation(t_acc[:], t_s[:], Act.Exp, scale=0.5)
                nc.scalar.activation(t_s[:], t_acc[:], Act.Ln, bias=1.0)
                nc.scalar.activation(t_acc[:], t_s[:], Act.Exp, scale=-1.0)

                t_u = lp.tile([P, N], dt, tag="t_u")
                nc.vector.tensor_scalar(
                    out=t_u[:], in0=t_uidb[:], scalar1=t_uidl[:, 0:1],
                    scalar2=None, op0=Alu.is_equal,
                )
                t_w = lp.tile([P, N], dt, tag="t_w")
                nc.gpsimd.tensor_mul(t_w[:], t_u[:], t_acc[:])

                # wT[m', l] per m-block via PE transpose
                t_wT = lp.tile([P, N], dt, tag="t_wT")
                for mb in range(N // P):
                    ps_t = ps.tile([P, P], dt, tag="pmain")
                    nc.tensor.transpose(
                        ps_t[:], t_w[:, mb * P:(mb + 1) * P], t_eye[:]
                    )
                    nc.scalar.copy(t_wT[:, mb * P:(mb + 1) * P], ps_t[:])

                # main accumulation + gate + store
                for mb in range(N // P):
                    t_awm = op_.tile([1, P * C], dt, tag="t_awm")
                    nc.sync.dma_start(t_awm[:], awt[mb:mb + 1, :])
                    ps_m = ps.tile([P, P * C], dt, tag="pmain")
                    for j in range(4):
                        sl = slice(j * 512, (j + 1) * 512)
                        nc.tensor.matmul(
                            ps_m[:, sl],
                            t_wT[:, mb * P:(mb + 1) * P],
                            t_D[:, sl],
                            start=True, stop=False,
                        )
                        nc.tensor.matmul(
                            ps_m[:, sl],
                            t_ones[:],
                            t_awm[0:1, sl],
                            start=False, stop=False,
                        )
                        nc.tensor.matmul(
                            ps_m[:, sl],
                            t_negaT[:, lb * P:(lb + 1) * P],
                            t_E[:],
                            start=False, stop=True,
                        )
                    t_out = op_.tile([P, P * C], dt, tag="t_out")
                    ub = t_u[:, mb * P:(mb + 1) * P]
                    u3 = ap3(ub, [list(ub.ap[1]), [0, C]])
                    p2 = ps_m[:, :]
                    p3 = ap3(p2, [[C, P], [1, C]])
                    o2 = t_out[:, :]
                    o3 = ap3(o2, [[C, P], [1, C]])
                    nc.vector.tensor_mul(o3, p3, u3)
                    nc.sync.dma_start(
                        plm_o[lb * P:(lb + 1) * P, mb * P:(mb + 1) * P, :],
                        t_out[:],
                    )
    nc.compile()
    return nc


def _prep_inputs(ref_pos, ref_mask, ref_element, ref_charge, ref_atom_name_chars,
                 ref_space_uid, W_feats, W_off, W_inv, W_mask):
    f4 = np.float32
    pos = np.asarray(ref_pos, f4)[0]                      # [N,3]
    uidf = np.asarray(ref_space_uid)[0].astype(f4)        # [N]
    a = pos @ np.asarray(W_off, f4)                       # [N,C]
    aw = a + np.asarray(W_mask, f4)[0]                    # [N,C]
    awt = np.ascontiguousarray(aw.reshape(-1).reshape(C, N))
    D = np.kron(np.eye(P, dtype=f4), np.asarray(W_inv, f4))   # [P, P*C]
    E = np.tile(np.eye(C, dtype=f4), (1, 512 // C))           # [C,512]
    feats = np.concatenate([
        pos,
        np.asarray(ref_mask, f4)[0][:, None],
        np.asarray(ref_element, f4)[0],
        np.asarray(ref_charge, f4)[0][:, None],
        np.asarray(ref_atom_name_chars, f4)[0].reshape(N, 256),
        uidf[:, None],
    ], axis=1)                                            # [N,390]
    fTp = np.zeros((512, N), f4)
    fTp[:C_IN] = feats.T
    wfp = np.zeros((512, C_ATOM), f4)
    wfp[:C_IN] = np.asarray(W_feats, f4)
    posm_b = np.ascontiguousarray(
        np.broadcast_to(pos.T[:, None, :], (3, P, N)), f4)
    uid_b = np.ascontiguousarray(np.broadcast_to(uidf[None, :], (P, N)), f4)
    ones1 = np.ones((1, P), f4)
    eye = np.eye(P, dtype=f4)
    wf4 = np.ascontiguousarray(wfp.reshape(4, P, C_ATOM))

    in_maps = []
    for i in range(NCORES):
        l0 = i * NL
        in_maps.append({
            "posm_b": posm_b,
            "uid_b": uid_b,
            "posl": np.ascontiguousarray(pos[l0:l0 + NL]),
            "uidl": np.ascontiguousarray(uidf[l0:l0 + NL, None]),
            "awt": awt,
            "negaT": np.ascontiguousarray((-a[l0:l0 + NL]).T),
            "dpat": D,
            "epat": E,
            "ones1": ones1,
            "eye": eye,
            "featsT": np.ascontiguousarray(
                fTp[:, l0:l0 + NL].reshape(4, P, NL)),
            "wfeats": wf4,
        })
    return in_maps


def kernel(**inputs):
    from concourse.bass_utils import run_bass_kernel_spmd

    if "nc" not in _CACHE:
        _CACHE["nc"] = _build_program()
    nc = _CACHE["nc"]
    in_maps = _prep_inputs(**inputs)
    res = run_bass_kernel_spmd(nc, in_maps, list(range(NCORES))).results
    cl = np.concatenate([r["cl_o"] for r in res], axis=0)[None]          # [1,N,128]
    plm = np.concatenate([r["plm_o"] for r in res], axis=0)[None]        # [1,N,N,C]
    return cl.astype(np.float32), plm.astype(np.float32)
